# revision 25
# baseline (speedup 1.0000x reference)
"""Trainium2 Bass kernel for pre-LN multi-head self-attention.

One batch element per core (8 cores, data parallel). Host fuses weights:
M_h = Wk_h^T Wq_h (scores become xn M xn^T) and VO_h = W_o_h W_v_h (out-proj
folded into V). Per core:

  Phase 1 (interleaved): LN -> xnT fp16; V_h = xn VO_h^T -> vt[h] bf16 with a
    ones column (AV matmul then emits softmax row-sums for free); G^T_h =
    M_h^T xn^T -> gT[h] fp16 for ALL heads (the ib2=0 half rides inside the
    LN loop, the rest right after) so the attention phase never waits on G.
  Phase 2: flat (h, ib) block stream; per block 16x [scores matmul fp16 ->
    f32 PSUM; exp on scalar engine (constant -75 shift) -> e bf16]. AV
    chunks flow through a GLOBAL pend queue of depth 5 that crosses block
    boundaries, so the tensor engine always has AV work to interleave with
    scores while exp latency drains -- no per-block pipeline drain. po
    accumulates [i, v | rowsum] in PSUM per (h, ib); its evacuation +
    normalize (reciprocal + tensor_scalar into y) run entirely on the DVE
    (keeping the scalar engine pure-exp in phase 2); out rows DMA as soon
    as the last head finishes them. The last block drains eagerly and
    pipelines its finish per-k with DMAs spread over two queues.

dtypes: fp16 scores side (bf16 logit noise costs 1.4e-2 rel err; fp16
2.4e-3), bf16 exp/V side (exp needs bf16 exponent range).
"""

import numpy as np

import concourse.bass as bass
import concourse.mybir as mybir
import concourse.tile as tile
from concourse import bacc
from concourse.bass_utils import run_bass_kernel_spmd

F32 = mybir.dt.float32
BF16 = mybir.dt.bfloat16
F16 = mybir.dt.float16

N_CORES = 8
N = 2048
DIM = 256
H = 8
EXP_SHIFT = 75.0

NT = N // 128
DC = DIM // 128
IB = N // 512
JP = NT // 2
VW = 258
PEND = 5


def build_nc_v24():
    nc = bacc.Bacc("TRN2", target_bir_lowering=False, debug=False,
                   num_devices=N_CORES)
    x_d = nc.dram_tensor("x", [N, DIM], F32, kind="ExternalInput")
    wq_d = nc.dram_tensor("wqkvT", [DIM, 2 * H * DIM], F16, kind="ExternalInput")
    id_d = nc.dram_tensor("ident", [128, 128], F32, kind="ExternalInput")
    out_d = nc.dram_tensor("out", [N, DIM], F32, kind="ExternalOutput")

    with tile.TileContext(nc) as tc:
        with (
            tc.tile_pool(name="singles", bufs=1) as singles,
            tc.tile_pool(name="xin", bufs=6) as xin,
            tc.tile_pool(name="lnst", bufs=6) as lnst,
            tc.tile_pool(name="etp", bufs=10) as etp,
            tc.tile_pool(name="tmpp", bufs=3) as tmpp,
            tc.tile_pool(name="pocp", bufs=2) as pocp,
            tc.tile_pool(name="rbp", bufs=4) as rbp,
            tc.tile_pool(name="ps_sc", bufs=2, space="PSUM") as ps_sc,
            tc.tile_pool(name="ps_acc", bufs=1, space="PSUM") as ps_acc,
        ):
            ident = singles.tile([128, 128], F32, tag="ident")
            nc.sync.dma_start(ident[:], id_d.ap()[:, :])
            eps_t = singles.tile([128, 1], F32, tag="eps")
            nc.vector.memset(eps_t, 1e-5)
            shift_t = singles.tile([128, 1], F32, tag="shift")
            nc.vector.memset(shift_t, -EXP_SHIFT)
            # warm the Exp activation table during the ramp: otherwise its
            # 1.28us ACT_TABLE_LOAD fires on the scalar queue right at the
            # phase-1->2 transition, stalling PE behind the copy backlog.
            warm = singles.tile([128, 1], F32, tag="warm")
            nc.scalar.activation(
                out=warm[:], in_=shift_t[:, 0:1],
                func=mybir.ActivationFunctionType.Exp,
                bias=shift_t[:, 0:1], scale=1.0)

            wqs = [[singles.tile([128, 2048], F16, tag=f"wq{dc}_{s}",
                                 name=f"wq{dc}_{s}") for s in range(2)]
                   for dc in range(DC)]
            xnT = [singles.tile([128, N], F16, tag=f"xnT{dc}", name=f"xnT{dc}")
                   for dc in range(DC)]
            vt = [singles.tile([128, NT, VW], BF16, tag=f"vt{h}", name=f"vt{h}")
                  for h in range(H)]
            gTa = [singles.tile([128, DC, N], F16, tag=f"gT{h}", name=f"gT{h}")
                   for h in range(H)]
            y = singles.tile([128, NT, DIM], F32, tag="y")

            def emit_ln(tcn):
                xt = xin.tile([128, DIM], F32, tag="xt")
                nc.sync.dma_start(xt[:], x_d.ap()[tcn * 128:(tcn + 1) * 128, :])
                stats = lnst.tile([128, 6], F32, tag="stats")
                nc.vector.bn_stats(out=stats[:], in_=xt[:])
                mv = lnst.tile([128, 2], F32, tag="mv")
                nc.vector.bn_aggr(out=mv[:], in_=stats[:])
                nc.scalar.activation(
                    out=mv[:, 1:2], in_=mv[:, 1:2],
                    func=mybir.ActivationFunctionType.Sqrt,
                    bias=eps_t[:, 0:1], scale=1.0)
                nc.vector.reciprocal(out=mv[:, 1:2], in_=mv[:, 1:2])
                # single-op normalize: the shortest cross-engine chain wins
                # over engine balance here (phase 1 is latency-bound).
                nc.vector.tensor_scalar(
                    out=xt[:], in0=xt[:], scalar1=mv[:, 0:1], scalar2=mv[:, 1:2],
                    op0=mybir.AluOpType.subtract, op1=mybir.AluOpType.mult)
                for dc in range(DC):
                    pst = ps_sc.tile([128, 2, 512], F32, tag="sc", name="pst")
                    nc.tensor.transpose(
                        pst[:, 0, :128], xt[:, dc * 128:(dc + 1) * 128], ident[:])
                    nc.vector.tensor_copy(
                        out=xnT[dc][:, tcn * 128:(tcn + 1) * 128],
                        in_=pst[:, 0, :128])

            vcopy_idx = [0]

            def emit_vbuild(tc2):
                # V-build PSUMs live in the acc pool (its 4 banks are idle
                # until attention starts), taking pressure off the 2 "sc"
                # slots that pace the LN-transpose/G-build rotation.
                for g in range(2):  # head-pair groups (2 pairs per acc tile)
                    ps = ps_acc.tile([128, 4, 512], F32, tag="acc", name="psv")
                    for pi in range(2):
                        p = 2 * g + pi
                        for half in range(2):
                            tcn = tc2 * 2 + half
                            for dc in range(DC):
                                nc.tensor.matmul(
                                    ps[:, 2 * pi + half, :],
                                    xnT[dc][:, tcn * 128:(tcn + 1) * 128],
                                    wqs[dc][1][:, p * 512:(p + 1) * 512],
                                    start=(dc == 0), stop=(dc == DC - 1))
                        for s in range(2):
                            # V copies split ~2:1 scalar:vector so neither
                            # queue paces phase 1.
                            dst = vt[2 * p + s][:, tc2 * 2:tc2 * 2 + 2, 0:256]
                            src = ps[:, 2 * pi:2 * pi + 2,
                                     s * 256:(s + 1) * 256]
                            if vcopy_idx[0] % 3 != 2:
                                nc.scalar.copy(out=dst, in_=src)
                            else:
                                nc.vector.tensor_copy(out=dst, in_=src)
                            vcopy_idx[0] += 1

            def emit_gbuild_half(h, ib2):
                gT = gTa[h]
                for mc in range(DC):
                    ps = ps_sc.tile([128, 2, 512], F32, tag="sc", name="psg")
                    for half in range(2):
                        jb = ib2 * 2 + half
                        for dc in range(DC):
                            nc.tensor.matmul(
                                ps[:, half, :],
                                wqs[dc][0][:, h * DIM + mc * 128:
                                           h * DIM + (mc + 1) * 128],
                                xnT[dc][:, jb * 512:(jb + 1) * 512],
                                start=(dc == 0), stop=(dc == DC - 1))
                    dst = gT[:, mc, ib2 * 1024:(ib2 + 1) * 1024]
                    # second halves drain at the phase-2 entry: keep them off
                    # the scalar queue so the first blocks' exps never queue
                    # behind them.
                    if ib2 == 0 and (h + mc) % 2 == 0:
                        nc.scalar.copy(out=dst, in_=ps[:, :, :])
                    else:
                        nc.vector.tensor_copy(out=dst, in_=ps[:, :, :])

            for h in range(H):
                nc.vector.memset(vt[h][:, :, 256:257], 1.0)

            # the ln(0)/ln(1) -> xnT chains gate the first V build; pin them
            # to the front of every engine's static schedule so the compile-
            # time scheduler can't batch later bn_stats ahead of them.
            with tc.high_priority():
                emit_ln(0)
                emit_ln(1)
            for dc in range(DC):  # VO weights behind x0/x1 (V build first)
                nc.sync.dma_start(
                    wqs[dc][1][:],
                    wq_d.ap()[dc * 128:(dc + 1) * 128, 2048:4096])
            for tc2 in range(1, NT // 2):
                emit_ln(tc2 * 2)
                if tc2 == 1:
                    for dc in range(DC):  # M weights behind x2
                        nc.sync.dma_start(
                            wqs[dc][0][:],
                            wq_d.ap()[dc * 128:(dc + 1) * 128, 0:2048])
                emit_ln(tc2 * 2 + 1)
                emit_vbuild(tc2 - 1)
                if tc2 >= 4:  # G first halves once token chunks 0..7 exist
                    emit_gbuild_half(2 * (tc2 - 4), 0)
                    emit_gbuild_half(2 * (tc2 - 4) + 1, 0)
            emit_vbuild(NT // 2 - 1)
            for h in range(H):
                emit_gbuild_half(h, 1)

            # ---- Phase 2: flat block stream with a global AV pend queue ----
            po_cur = [None]
            pend = []

            def emit_finish(h, ib, last=False):
                # po evacuation + normalize, entirely on the DVE: the scalar
                # engine stays pure-exp so the next block's exps are never
                # delayed behind copies. The last block pipelines per-k and
                # spreads the output DMAs over two queues to shorten the
                # kernel tail.
                po = po_cur[0]
                poc = pocp.tile([128, 4, 257], F32, tag="poc")
                rb = rbp.tile([128, 4, 1], F32, tag="rb")
                # last-head evacuations ride the scalar engine: its exp
                # stream is ending right then, while the eager drain of the
                # final blocks otherwise piles ~8us of finish work onto the
                # DVE at the kernel tail (plain COPY loads no act table).
                if not last:
                    cp = nc.scalar.copy if h == H - 1 else (
                        lambda out, in_: nc.vector.tensor_copy(out=out, in_=in_))
                    cp(out=poc[:, 0:2, :], in_=po[:, 0:2, 0:257])
                    cp(out=poc[:, 2:4, :], in_=po[:, 2:4, 0:257])
                    nc.vector.reciprocal(
                        out=rb[:, :, :], in_=poc[:, :, 256:257])
                for k in range(4):
                    i128 = ib * 4 + k
                    if last:
                        nc.scalar.copy(
                            out=poc[:, k:k + 1, :], in_=po[:, k:k + 1, 0:257])
                        nc.vector.reciprocal(
                            out=rb[:, k:k + 1, :], in_=poc[:, k:k + 1, 256:257])
                    if h == 0:
                        nc.vector.tensor_scalar(
                            out=y[:, i128, :], in0=poc[:, k, 0:256],
                            scalar1=rb[:, k, 0:1], scalar2=None,
                            op0=mybir.AluOpType.mult)
                    else:
                        tmpt = tmpp.tile([128, DIM], F32, tag="tmp")
                        nc.vector.tensor_scalar(
                            out=tmpt[:], in0=poc[:, k, 0:256],
                            scalar1=rb[:, k, 0:1], scalar2=None,
                            op0=mybir.AluOpType.mult)
                        nc.vector.tensor_add(
                            out=y[:, i128, :], in0=y[:, i128, :],
                            in1=tmpt[:])
                    if h == H - 1:
                        eng = nc.scalar if (last and k % 2) else nc.sync
                        eng.dma_start(
                            out_d.ap()[i128 * 128:(i128 + 1) * 128, :],
                            y[:, i128, :])

            def emit_one():
                h, ib, jp, e_t = pend.pop(0)
                if jp == 0:
                    po_cur[0] = ps_acc.tile([128, 4, 512], F32, tag="acc",
                                            name="po")
                po = po_cur[0]
                vth = vt[h]
                for half in range(2):
                    jc = jp * 2 + half
                    for k in range(4):
                        nc.tensor.matmul(
                            po[:, k, 0:257],
                            e_t[:, half, k * 128:(k + 1) * 128],
                            vth[:, jc, 0:257],
                            start=(jp == 0 and half == 0), stop=(jc == NT - 1))
                if jp == JP - 1:
                    emit_finish(h, ib, last=(h == H - 1 and ib == IB - 1))

            for h in range(H):
                for ib in range(IB):
                    last = (h == H - 1 and ib == IB - 1)
                    for jp in range(JP):
                        ps = ps_sc.tile([128, 2, 512], F32, tag="sc",
                                        name="pss")
                        for half in range(2):
                            jc = jp * 2 + half
                            for dc in range(DC):
                                nc.tensor.matmul(
                                    ps[:, half, :],
                                    gTa[h][:, dc, jc * 128:(jc + 1) * 128],
                                    xnT[dc][:, ib * 512:(ib + 1) * 512],
                                    start=(dc == 0), stop=(dc == DC - 1))
                        e_t = etp.tile([128, 2, 512], BF16, tag="et")
                        nc.scalar.activation(
                            out=e_t[:, :, :], in_=ps[:, :, :],
                            func=mybir.ActivationFunctionType.Exp,
                            bias=shift_t[:, 0:1], scale=1.0)
                        pend.append((h, ib, jp, e_t))
                        # the last block drains eagerly so the kernel tail is
                        # just one AV chunk + finish, not a full pend flush.
                        depth = 1 if last else PEND
                        while len(pend) > depth:
                            emit_one()
            while pend:
                emit_one()

    nc.compile()
    return nc


_NC_CACHE = {}


def _get_nc():
    if "v24" not in _NC_CACHE:
        _NC_CACHE["v24"] = build_nc_v24()
    return _NC_CACHE["v24"]


def _prep_in_maps(x, w_qkv, w_out, gamma, beta):
    x = np.ascontiguousarray(np.asarray(x), dtype=np.float32)
    w_qkv = np.asarray(w_qkv, dtype=np.float32)
    w_out = np.asarray(w_out, dtype=np.float32)
    gamma = np.asarray(gamma, dtype=np.float32)
    beta = np.asarray(beta, dtype=np.float32)
    assert x.shape == (N_CORES, N, DIM), x.shape
    if np.abs(beta).max() != 0.0:
        raise NotImplementedError("nonzero LayerNorm beta not supported")
    w_eff = w_qkv * gamma[None, :]
    M = np.concatenate([
        w_eff[H * DIM + h * DIM:H * DIM + (h + 1) * DIM, :].T @
        w_eff[h * DIM:(h + 1) * DIM, :]
        for h in range(H)
    ], axis=1)
    w_vo = np.concatenate([
        w_out[:, h * DIM:(h + 1) * DIM] @
        w_eff[2 * H * DIM + h * DIM:2 * H * DIM + (h + 1) * DIM, :]
        for h in range(H)
    ], axis=0)
    wqkvT = np.empty((DIM, 2 * H * DIM), np.float32)
    wqkvT[:, :H * DIM] = M
    wqkvT[:, H * DIM:] = w_vo.T
    wqkvT = np.ascontiguousarray(wqkvT).astype(np.float16)
    ident = np.eye(128, dtype=np.float32)
    return [
        {"x": np.ascontiguousarray(x[i]), "wqkvT": wqkvT, "ident": ident}
        for i in range(N_CORES)
    ]


def run(inputs, trace=False):
    nc = _get_nc()
    in_maps = _prep_in_maps(**inputs)
    res = run_bass_kernel_spmd(nc, in_maps, core_ids=list(range(N_CORES)),
                               trace=trace)
    out = np.stack([res.results[i]["out"] for i in range(N_CORES)], axis=0)
    return out, res


def kernel(**inputs) -> np.ndarray:
    out, _ = run(inputs, trace=False)
    return out
